# revision 22
# baseline (speedup 1.0000x reference)
"""Trainium2 Bass kernel for nn_MoEConnectionProcessor.

Data-parallel over cells: 8 cores x 2560 padded cells (19683 real).
Per core the cell range is processed in 40 superblocks of 64 cells
(= 13 subtiles of 128 edges, since 64*26 = 13*128 = 1664).

Design notes:
  - all mask/count/scale math is folded host-side into the staircase
    aggregation constants (B_ld, B_f); no on-device counts/reciprocals.
  - neighbor data ships in two host-prepared layouts (edge-major for
    aggregation stationaries, d-major for the projection stationary).
  - the per-edge cur@Wm1 term is added by a staircase matmul with a
    65th all-ones K row whose moving-operand row holds b_msg.
  - superblocks run in pairs sharing the S65 staircase LDWEIGHTS;
    aggregation matmuls of pair-group k are emitted after the
    projections of group k+1 so relu latency never stalls the PE;
    relu evacuation alternates DVE/ACT.
  - CNF is reparametrized as u = s/DT (one f32 add per Euler step);
    1/DT is folded into the distant staircase scales, DT into the ACT
    tanh input scale and the distant gate broadcast constant.
  - gating/expert/CNF/mix work is queued and drained one item per
    half-pair; per-chunk aggregate tiles avoid write-after-read
    serialization against the main loop.
  - constants are shipped in two packed buffers (one early, one late)
    to minimize DMA dispatch overhead at startup.
"""

import numpy as np
import ml_dtypes
from contextlib import ExitStack

N_CELLS, K, D, HG = 19683, 26, 128, 64
NCORES = 8
NS = 2560                 # padded cells per core
SBC = 64                  # cells per superblock
NSB = NS // SBC           # 40 superblocks
NSUB = 13                 # subtiles (128 edges) per superblock
EPB = NSUB * 128          # 1664 edges per superblock
E = NS * K                # 66560 edges per core
NSUBT = NS * K // 128     # 520 subtiles per core
CHUNK = 512
NCHUNK = NS // CHUNK      # 5
SB_PER_CHUNK = CHUNK // SBC  # 8
CNF_STEPS, DTC = 3, 0.1

bf16 = ml_dtypes.bfloat16

# first local cell of each subtile class
CB_LOC = [(chi * 128) // K for chi in range(NSUB)]

# packed constant layouts: (name, partitions, cols)
PACKA = [("Wm1", 128, 128), ("Wm2", 128, 128), ("S65", 65, NSUB * 128),
         ("curTb", 128, NS), ("Wg1", 128, HG), ("Wg2", HG, 3),
         ("OH3", 3, 384), ("ONES3", 3, 1), ("ONES13", 1, 3)]
PACKL = [("B_ld", 128, NSUBT * 12), ("B_f", 128, NSUBT * 6),
         ("Wl1", 128, 128), ("Wl2", 128, 128), ("Wu1", 128, 128),
         ("Wu2", 128, 128), ("Wc1", 128, 128), ("Wc2", 128, 128),
         ("u0b", 128, NS)]
PACKB = [("b_local", 128, 1), ("b_upd", 128, 1), ("b_cnf", 128, 1),
         ("b_g1", HG, 1), ("b_g2", 3, 1)]


def _offsets(layout):
    offs, o = {}, 0
    for nm, p, c in layout:
        offs[nm] = (o, p, c)
        o += c
    return offs, o


OFFA, NA = _offsets(PACKA)
OFFL, NL = _offsets(PACKL)
OFFB, NB = _offsets(PACKB)


def _consts():
    c = {}
    s65 = np.zeros((65, NSUB * 128), np.float32)
    idx = np.arange(NSUB * 128)
    s65[idx // K, idx] = 1.0
    s65[64, :] = 1.0
    c["S65"] = s65
    oh = np.zeros((3, 3 * 128), np.float32)
    for m in range(3):
        oh[m, m * 128:(m + 1) * 128] = 1.0 if m < 2 else DTC
    c["OH3"] = oh
    c["ONES3"] = np.ones((3, 1), np.float32)
    c["ONES13"] = np.ones((1, 3), np.float32)
    return c


CONSTS = _consts()


def _build_bass():
    import concourse.bass as bass
    import concourse.tile as tile
    from concourse import bacc, mybir
    from collections import deque

    f32, bft = mybir.dt.float32, mybir.dt.bfloat16
    f8 = mybir.dt.float8e4
    AF = mybir.ActivationFunctionType
    OP = mybir.AluOpType

    nc = bacc.Bacc("TRN2", target_bir_lowering=False, debug=False,
                   num_devices=NCORES)

    def din(name, shape, dt):
        return nc.dram_tensor(name, shape, dt, kind="ExternalInput").ap()

    nbr_nat = din("nbr_nat", [128, NSUBT * D], bft)   # edge-major subtiles
    natT = din("natT", [128, E], f8)                  # d-major (pretransposed)
    wm28_d = din("Wm2_8", [128, 128], f8)
    packA_d = din("packA", [128, NA], bft)
    packL_d = din("packL", [128, NL], bft)
    packB_d = din("packB", [128, NB], f32)
    u0f_d = din("u0f", [D, NS], f32)                  # cur.T / DT
    bmsg_d = din("bmsg_tiled", [1, NSB * 128], bft)
    outT = nc.dram_tensor("outT", [D, NS], f32, kind="ExternalOutput").ap()

    with tile.TileContext(nc) as tc, ExitStack() as ctx:
        const = ctx.enter_context(tc.tile_pool(name="const", bufs=1))
        big = ctx.enter_context(tc.tile_pool(name="big", bufs=1))
        nat_p = ctx.enter_context(tc.tile_pool(name="nat", bufs=6))
        natT_p = ctx.enter_context(tc.tile_pool(name="natT", bufs=6))
        msgs_p = ctx.enter_context(tc.tile_pool(name="msgs", bufs=4))
        s2tmp = ctx.enter_context(tc.tile_pool(name="s2tmp", bufs=2))
        ps_proj = ctx.enter_context(tc.tile_pool(name="ps_proj", bufs=4,
                                                 space="PSUM"))
        ps_agg = ctx.enter_context(tc.tile_pool(name="ps_agg", bufs=2,
                                                space="PSUM"))
        ps_s2 = ctx.enter_context(tc.tile_pool(name="ps_s2", bufs=2,
                                               space="PSUM"))

        # ---------- packed constants ----------
        packA = const.tile([128, NA], bft)
        nc.sync.dma_start(packA[:], packA_d[:])
        packB = const.tile([128, NB], f32)
        nc.sync.dma_start(packB[:], packB_d[:])
        wm28 = const.tile([128, 128], f8)
        nc.sync.dma_start(wm28[:], wm28_d[:])
        packL = const.tile([128, NL], bft)   # DMA'd after first pair
        u0f = const.tile([D, NS], f32)       # DMA'd after first pair

        def vA(nm):
            o, p, c = OFFA[nm]
            return packA[0:p, o:o + c]

        def vL(nm):
            o, p, c = OFFL[nm]
            return packL[0:p, o:o + c]

        def vB(nm):
            o, p, c = OFFB[nm]
            return packB[0:p, o:o + c]

        wt = {k: vA(k) for k in ["Wm1", "Wm2"]}
        for k in ["Wl1", "Wl2", "Wu1", "Wu2", "Wc1", "Wc2"]:
            wt[k] = vL(k)
        wg1, wg2 = vA("Wg1"), vA("Wg2")
        s65, oh3 = vA("S65"), vA("OH3")
        ones3, ones13 = vA("ONES3"), vA("ONES13")
        curTb = vA("curTb")
        u0b, bld, bfc = vL("u0b"), vL("B_ld"), vL("B_f")
        bias = {k: vB(k) for k in ["b_local", "b_upd", "b_cnf", "b_g1",
                                   "b_g2"]}

        # persistent activation buffers (aggregates split per chunk to
        # avoid write-after-read serialization with deferred items)
        aggld_c = [big.tile([128, SB_PER_CHUNK * 128], bft, name=f"agl{i}",
                            tag=f"agl{i}") for i in range(NCHUNK)]
        aggf_c = [big.tile([128, SB_PER_CHUNK * 64], bft, name=f"agf{i}",
                           tag=f"agf{i}") for i in range(NCHUNK)]
        cpm = big.tile([65, NSB * 128], bft)       # rows 0-63 cur@Wm1, 64 bias
        localT = big.tile([128, NS], bft)
        funcT = big.tile([128, NS], bft)
        hT = big.tile([HG, NS], bft)
        e3 = big.tile([3, NS], bft)
        recf = big.tile([1, NS], f32)
        recb = big.tile([1, NS], bft)
        gates = big.tile([3, NS], bft)
        u_f = [big.tile([128, NS], f32, name=f"uf{i}", tag=f"uf{i}")
               for i in range(2)]
        u_b = big.tile([128, NS], bft)

        # ---------- main-loop building blocks ----------
        def cpm_mm(t2):
            pc = ps_s2.tile([SBC, 256], f32, tag="p")
            for h in range(2):
                t = t2 + h
                nc.tensor.matmul(pc[:, h * 128:(h + 1) * 128],
                                 curTb[:, t * SBC:(t + 1) * SBC],
                                 wt["Wm1"][:], start=True, stop=True)
            nc.scalar.copy(cpm[0:SBC, t2 * 128:(t2 + 2) * 128], pc[:])

        GROUPS = [(0, 4), (4, 4), (8, 4), (12, 1)]

        class PairState:
            pass

        def pair_open(t):
            st = PairState()
            st.t = t
            st.natT, st.nat, st.msgs = [], [], []
            for u in (t, t + 1):
                natT_t = natT_p.tile([128, EPB], f8, tag="natT",
                                     name=f"natT{u}")
                nc.sync.dma_start(natT_t[:], natT[:, u * EPB:(u + 1) * EPB])
                st.natT.append(natT_t)
                nat_t = nat_p.tile([128, NSUB, 128], bft, tag="nat",
                                   name=f"nat{u}")
                nc.sync.dma_start(
                    nat_t[:], nbr_nat[:, u * EPB:(u + 1) * EPB].rearrange(
                        "p (s d) -> p s d", d=128))
                st.nat.append(nat_t)
                st.msgs.append(msgs_p.tile([128, EPB], bft, tag="msgs",
                                           name=f"msgs{u}"))
            st.pagg2 = ps_agg.tile([128, 384], f32, tag="pagg",
                                   name=f"pagg{t}")
            st.pagg = [st.pagg2[:, 0:192], st.pagg2[:, 192:384]]
            return st

        def pair_projq(st, gi):
            g0, gn = GROUPS[gi]
            st_pm = []
            for h in range(2):
                st_pm.append(ps_proj.tile([128, 512], f32, tag="p",
                                          name=f"pm{h}"))
            for i in range(gn):
                s = g0 + i
                csl = slice(i * 128, (i + 1) * 128)
                for h in range(2):
                    nc.tensor.matmul(st_pm[h][:, csl],
                                     st.natT[h][:, s * 128:(s + 1) * 128],
                                     wm28[:], start=True, stop=False)
                for h in range(2):
                    mm = nc.tensor.matmul(
                        st_pm[h][:, csl], s65[:, s * 128:(s + 1) * 128],
                        cpm[:, (st.t + h) * 128:(st.t + h + 1) * 128],
                        start=False, stop=True)
                    if h == 1:
                        mm.ins.ldweights = False
            sl = slice(g0 * 128, (g0 + gn) * 128)
            nc.vector.tensor_scalar(st.msgs[0][:, sl], st_pm[0][:, 0:gn * 128],
                                    0.0, None, OP.max)
            nc.scalar.activation(st.msgs[1][:, sl], st_pm[1][:, 0:gn * 128],
                                 AF.Relu)

        def pair_agg(st, gi):
            if gi == 0:
                nc.vector.memset(st.pagg2[:], 0.0)
            g0, gn = GROUPS[gi]
            for i in range(gn):
                s = g0 + i
                cb = CB_LOC[s]
                w = min(6, SBC - cb)
                last = s == NSUB - 1
                for h in range(2):
                    sg = (st.t + h) * NSUB + s
                    nc.tensor.matmul(st.pagg[h][:, 2 * cb:2 * cb + 2 * w],
                                     st.nat[h][:, s, :],
                                     bld[:, sg * 12:sg * 12 + 2 * w],
                                     start=False, stop=last)
                    nc.tensor.matmul(st.pagg[h][:, 128 + cb:128 + cb + w],
                                     st.msgs[h][:, s * 128:(s + 1) * 128],
                                     bfc[:, sg * 6:sg * 6 + w],
                                     start=False, stop=last)

        def pair_evac(st):
            for h in range(2):
                u = st.t + h
                ch, r = u // SB_PER_CHUNK, u % SB_PER_CHUNK
                nc.vector.tensor_copy(aggld_c[ch][:, r * 128:(r + 1) * 128],
                                      st.pagg[h][:, 0:128])
                nc.vector.tensor_copy(aggf_c[ch][:, r * 64:(r + 1) * 64],
                                      st.pagg[h][:, 128:192])

        # ---------- deferred work items ----------
        def agg_view(base_off, c0, n):
            ch = c0 // CHUNK
            r0 = (c0 % CHUNK) // SBC
            nr = n // SBC
            v = aggld_c[ch][:, r0 * 128 + base_off:(r0 + nr) * 128:2]
            return v.rearrange("p (t c) -> p t c", c=64)

        def gat_h():
            for ch in range(NCHUNK):
                sl = slice(ch * CHUNK, (ch + 1) * CHUNK)
                ph = ps_s2.tile([HG, CHUNK], f32, tag="p")
                mm = nc.tensor.matmul(ph[:], wg1[:], curTb[:, sl], start=True,
                                      stop=True)
                if ch:
                    mm.ins.ldweights = False
                nc.scalar.activation(hT[:, sl], ph[:], AF.Relu,
                                     bias=bias["b_g1"])

        def gat_e():
            for ch in range(NCHUNK):
                sl = slice(ch * CHUNK, (ch + 1) * CHUNK)
                pz = ps_s2.tile([3, CHUNK], f32, tag="p")
                mm = nc.tensor.matmul(pz[:], wg2[:], hT[:, sl], start=True,
                                      stop=True)
                if ch:
                    mm.ins.ldweights = False
                nc.scalar.activation(e3[:, sl], pz[:], AF.Exp,
                                     bias=bias["b_g2"])

        def gat_r():
            for ch in range(NCHUNK):
                sl = slice(ch * CHUNK, (ch + 1) * CHUNK)
                pgs = ps_s2.tile([1, CHUNK], f32, tag="p")
                mm = nc.tensor.matmul(pgs[:], ones3[:], e3[:, sl], start=True,
                                      stop=True)
                if ch:
                    mm.ins.ldweights = False
                nc.vector.reciprocal_approx_fast(recf[:, sl], pgs[:])
            nc.vector.tensor_copy(recb[:], recf[:])

        def gat_g():
            for ch in range(NCHUNK):
                sl = slice(ch * CHUNK, (ch + 1) * CHUNK)
                pr3 = ps_s2.tile([3, CHUNK], f32, tag="p")
                mm = nc.tensor.matmul(pr3[:], ones13[:], recb[:, sl],
                                      start=True, stop=True)
                if ch:
                    mm.ins.ldweights = False
                nc.vector.tensor_tensor(gates[:, sl], e3[:, sl], pr3[:],
                                        OP.mult)

        def item_local(c0, n):
            sl = slice(c0, c0 + n)
            pl = ps_s2.tile([128, CHUNK], f32, tag="p")
            nc.tensor.matmul(pl[:, 0:n], wt["Wl1"][:], curTb[:, sl],
                             start=True, stop=False)
            nc.tensor.matmul(
                pl[:, 0:n].rearrange("p (t c) -> p t c", c=64),
                wt["Wl2"][:], agg_view(0, c0, n), start=False, stop=True)
            nc.scalar.activation(localT[:, sl], pl[:, 0:n], AF.Tanh,
                                 bias=bias["b_local"])

        def item_func(c0, n):
            sl = slice(c0, c0 + n)
            ch = c0 // CHUNK
            f0 = (c0 % CHUNK)
            pf = ps_s2.tile([128, CHUNK], f32, tag="p")
            nc.tensor.matmul(pf[:, 0:n], wt["Wu1"][:], curTb[:, sl],
                             start=True, stop=False)
            nc.tensor.matmul(
                pf[:, 0:n].rearrange("p (t c) -> p t c", c=64),
                wt["Wu2"][:],
                aggf_c[ch][:, f0:f0 + n].rearrange("p (t c) -> p t c", c=64),
                start=False, stop=True)
            nc.scalar.activation(funcT[:, sl], pf[:, 0:n], AF.Tanh,
                                 bias=bias["b_upd"])

        def item_cnf(c0, n, step):
            sl = slice(c0, c0 + n)
            src_b = u0b if step == 0 else u_b
            src_f = u0f if step == 0 else u_f[(step - 1) % 2]
            dst_f = u_f[step % 2]
            pp = ps_s2.tile([128, CHUNK], f32, tag="p")
            nc.tensor.matmul(pp[:, 0:n], wt["Wc1"][:], src_b[:, sl],
                             start=True, stop=False)
            nc.tensor.matmul(
                pp[:, 0:n].rearrange("p (t c) -> p t c", c=64),
                wt["Wc2"][:], agg_view(1, c0, n), start=False, stop=True)
            th = s2tmp.tile([128, CHUNK], f32, tag="th")
            nc.scalar.activation(th[:, 0:n], pp[:, 0:n], AF.Tanh,
                                 bias=bias["b_cnf"], scale=DTC)
            nc.vector.tensor_tensor(dst_f[:, sl], src_f[:, sl], th[:, 0:n],
                                    OP.add)
            if step < CNF_STEPS - 1:
                nc.vector.tensor_copy(u_b[:, sl], dst_f[:, sl])

        def item_mix(c0, n):
            sl = slice(c0, c0 + n)
            uf_last = u_f[(CNF_STEPS - 1) % 2]
            acc = s2tmp.tile([128, CHUNK], f32, tag="acc")
            tmp = s2tmp.tile([128, CHUNK], f32, tag="tmp")
            experts = [localT[:, sl], funcT[:, sl], uf_last[:, sl]]
            for m in range(3):
                p = ps_s2.tile([128, CHUNK], f32, tag="p")
                nc.tensor.matmul(p[:, 0:n], oh3[:, m * 128:(m + 1) * 128],
                                 gates[:, sl], start=True, stop=True)
                if m == 0:
                    nc.vector.tensor_tensor(acc[:, 0:n], experts[m],
                                            p[:, 0:n], OP.mult)
                else:
                    nc.vector.tensor_tensor(tmp[:, 0:n], experts[m],
                                            p[:, 0:n], OP.mult)
                    nc.vector.tensor_tensor(acc[:, 0:n], acc[:, 0:n],
                                            tmp[:, 0:n], OP.add)
            nc.sync.dma_start(outT[:, sl], acc[:, 0:n])

        def chunk_items(c0, n):
            return [lambda: item_local(c0, n), lambda: item_func(c0, n),
                    lambda: item_cnf(c0, n, 0), lambda: item_cnf(c0, n, 1),
                    lambda: item_cnf(c0, n, 2), lambda: item_mix(c0, n)]

        # cell ranges for deferred items: last chunk split so its first
        # half drains during the loop
        RANGES = [(0, 512), (512, 512), (1024, 512), (1536, 512),
                  (2048, 256), (2304, 256)]

        # ---------- driver ----------
        work_q = deque([gat_h, gat_e, gat_r, gat_g])
        NG = len(GROUPS)
        pend = deque()
        prev = None
        cpm_mm(0)
        nxt = pair_open(0)
        nc.sync.dma_start(packL[:], packL_d[:])
        nc.sync.dma_start(cpm[64:65, :], bmsg_d[:])
        nc.sync.dma_start(u0f[:], u0f_d[:])
        for t2 in range(0, NSB, 2):
            st = nxt
            if t2 + 2 < NSB:
                cpm_mm(t2 + 2)
                nxt = pair_open(t2 + 2)
            for gi in range(NG):
                pair_projq(st, gi)
                if len(pend) == 2:
                    pair_agg(*pend.popleft())
                pend.append((st, gi))
                if gi == 2 and prev is not None:
                    pair_evac(prev)
                    done = t2 * SBC
                    while RANGES and RANGES[0][0] + RANGES[0][1] <= done:
                        work_q.extend(chunk_items(*RANGES.pop(0)))
                if gi in (1, 2, 3) and work_q:
                    work_q.popleft()()
            prev = st
        while pend:
            pair_agg(*pend.popleft())
        pair_evac(st)
        while RANGES:
            work_q.extend(chunk_items(*RANGES.pop(0)))
        while work_q:
            work_q.popleft()()

    nc.compile()
    return nc


_NC_CACHE = None


def _get_nc():
    global _NC_CACHE
    if _NC_CACHE is None:
        _NC_CACHE = _build_bass()
    return _NC_CACHE


def _pack(layout, parts, np_dtype):
    n = sum(c for _, _, c in layout)
    out = np.zeros((128, n), np_dtype)
    o = 0
    for nm, p, c in layout:
        out[0:p, o:o + c] = parts[nm]
        o += c
    return out


def _prep_core_inputs(cur, nbr, conn, weights):
    """cur [NS, D] f32, nbr [NS, K, D] f32, conn [NS, K] i32 -> input map."""
    m = {}
    nf = nbr.reshape(E, D).astype(bf16)
    m["nbr_nat"] = np.ascontiguousarray(
        nf.reshape(NSUBT, 128, D).transpose(1, 0, 2)).reshape(128, NSUBT * D)
    f8np = ml_dtypes.float8_e4m3
    m["natT"] = np.ascontiguousarray(nbr.reshape(E, D).T).astype(f8np)
    m["Wm2_8"] = weights["W_msg"][D:].astype(f8np)
    ct = np.ascontiguousarray(cur.T).astype(np.float32)
    u0 = ct / DTC
    m["u0f"] = u0

    # host-side masks, counts and scales folded into staircases
    ctype = conn.reshape(E)
    ml = ctype == 0
    mf = ctype == 1
    md = ctype == 2
    cnt = lambda mm: np.maximum(mm.reshape(NS, K).sum(1).astype(np.float32),
                                1.0)
    cl, cf, cd = cnt(ml), cnt(mf), cnt(md)
    e = np.arange(E)
    s = e // 128
    p = e % 128
    c = e // K
    j = c - (s * 128) // K
    B_ld = np.zeros((128, NSUBT * 12), np.float32)
    B_ld[p, s * 12 + 2 * j] = ml / cl[c]
    B_ld[p, s * 12 + 2 * j + 1] = md / (cd[c] * DTC)
    B_f = np.zeros((128, NSUBT * 6), np.float32)
    B_f[p, s * 6 + j] = mf / cf[c]

    Wl, Wm, Wu, Wc = (weights["W_local"], weights["W_msg"],
                      weights["W_upd"], weights["W_cnf"])
    partsA = {"Wm1": Wm[:D], "Wm2": Wm[D:],
              "Wg1": weights["W_g1"], "Wg2": weights["W_g2"],
              "S65": CONSTS["S65"], "OH3": CONSTS["OH3"],
              "ONES3": CONSTS["ONES3"], "ONES13": CONSTS["ONES13"],
              "curTb": ct}
    m["packA"] = _pack(PACKA, partsA, bf16)
    m["packL"] = _pack(PACKL, {"u0b": u0, "B_ld": B_ld, "B_f": B_f,
                               "Wl1": Wl[:D], "Wl2": Wl[D:], "Wu1": Wu[:D],
                               "Wu2": Wu[D:], "Wc1": Wc[:D], "Wc2": Wc[D:]},
                      bf16)
    partsB = {"b_local": weights["b_local"].reshape(D, 1),
              "b_upd": weights["b_upd"].reshape(D, 1),
              "b_cnf": weights["b_cnf"].reshape(D, 1),
              "b_g1": weights["b_g1"].reshape(HG, 1),
              "b_g2": weights["b_g2"].reshape(3, 1)}
    m["packB"] = _pack(PACKB, partsB, np.float32)
    m["bmsg_tiled"] = np.tile(
        weights["b_msg"].reshape(1, D), (1, NSB)).astype(bf16)
    return m


def kernel(**inputs):
    from concourse.bass_utils import run_bass_kernel_spmd

    cur = np.asarray(inputs["current_state"], np.float32)
    nbr = np.asarray(inputs["neighbor_states"], np.float32)
    conn = np.asarray(inputs["conn_type"], np.int32)
    weights = {k: np.asarray(v, np.float32) for k, v in inputs.items()
               if k not in ("current_state", "neighbor_states", "conn_type")}

    npad = NCORES * NS
    cur_p = np.zeros((npad, D), np.float32)
    cur_p[:N_CELLS] = cur
    nbr_p = np.zeros((npad, K, D), np.float32)
    nbr_p[:N_CELLS] = nbr
    conn_p = np.full((npad, K), 3, np.int32)
    conn_p[:N_CELLS] = conn

    in_maps = []
    for c in range(NCORES):
        sl = slice(c * NS, (c + 1) * NS)
        in_maps.append(_prep_core_inputs(cur_p[sl], nbr_p[sl], conn_p[sl],
                                         weights))
    nc = _get_nc()
    res = run_bass_kernel_spmd(nc, in_maps, list(range(NCORES)))
    out = np.concatenate([res.results[c]["outT"].T.astype(np.float32)
                          for c in range(NCORES)], axis=0)
    return np.ascontiguousarray(out[:N_CELLS]).astype(np.float32)


if __name__ == "__main__":
    pass
